# revision 15
# baseline (speedup 1.0000x reference)
"""Trainium2 Bass kernel for nn_CSPN (sum-product network layer).

out[b,s] = logsumexp_n(prod[b,n] + log_softmax_n(gate[b,n,s]))
         = log(S1[b,s]) - log(S0[b,s]) + C1[b]
where prod[b,n] = lp1[b, n%32] + lp2[b, n//32]  (Gaussian leaf log-probs),
      C1[b] = per-row shift (exp(prod) ~ e^-90 underflows without it),
      S1 = sum_n exp(prod - C1) * exp(g),  S0 = sum_n exp(g).

Sharding: data-parallel over batch, 512 rows per core, no communication.

Per-core dataflow (v2, tuned from the 272us baseline trace):
- The kernel is HBM-DMA-bound: 67MB of gate reads at ~358 GB/s/core floor.
  The baseline lost ~30us to tile-boundary stalls (prep work head-blocking
  the ACT queue and gate-buffer WAR waits) plus a 17us serial epilogue.
- All leaf prep (means DMAs on the sync queue, leaf log-probs, stationary
  weights) is emitted UP FRONT; a deep 8-buffer gate pool rides out the
  startup while prep completes.
- ALL gate DMAs stream on the gpsimd SWDGE queue with fp32->bf16 cast
  during DMA (halves SBUF footprint; SWDGE packetizes 1KB descriptors
  into 4KB packets). One queue alone sustains ~350 GB/s.
- exp runs in 4 chunks per 32-row quarter; matmuls accumulate per-octet
  (n = 8p + i) into PSUM [32, 512] exactly as the baseline.
- The evac (Ln, diag extraction, +C1, transpose, output DMA) streams
  per quarter on ACT/DVE/sync, so no epilogue remains but the last
  quarter's short chain.
- C1[b] never round-trips DRAM: a TensorE transpose + 4 tiny
  indicator-stationary matmuls broadcast it to [32 s, 512 b] in PSUM.
"""

import sys

sys.path.insert(0, "/opt/trn_rl_repo")

import numpy as np

B = 4096
K = 32          # gaussians per region
S = 32          # gating outputs
N = K * K       # 1024 products
NCORES = 8
BC = B // NCORES    # 512 batch rows per core
P = 128
NT = BC // P        # 4 batch tiles per core
NG = P // 16        # 8 groups of 16 batch rows per tile
NI = N // P         # 8 n's per partition (octet)
NQ = NG // 2        # 4 quarters (32 batch rows) per tile

LOG2PI = float(np.log(2.0 * np.pi))
BCONST = -K * LOG2PI    # prod = -0.5*(raw1+raw2) + BCONST

_cache = {}


def _strided_cols(bass, ap, start, step, count):
    """AP selecting free columns start, start+step, ... of a [P, F] AP."""
    return bass.AP(
        tensor=ap.tensor,
        offset=ap.offset + start * ap.ap[-1][0],
        ap=[ap.ap[0], [step * ap.ap[-1][0], count]],
    )


def _patch_act_tables():
    """Make the table-load chooser use the combined exp+ln set so Exp and Ln
    activations don't ping-pong ~1.3us ACT_TABLE_LOADs between two sets.
    Set ids are positional, so contents are masked rather than reordered."""
    from concourse import bacc, hw_specs
    import concourse.mybir as mybir

    if getattr(bacc, "_act_tables_patched", False):
        return
    orig = hw_specs.get_activation_tables

    def patched(module_arch):
        tabs = orig(module_arch)
        AF = mybir.ActivationFunctionType
        both = {AF.Exp, AF.Ln}
        return {
            name: (fns - both if name != "natural_log_exp_and_others" else fns)
            for name, fns in tabs.items()
        }

    bacc.get_activation_tables = patched
    bacc._act_tables_patched = True


def _build():
    import concourse.bass as bass
    import concourse.mybir as mybir
    import concourse.tile as tile
    from concourse import bacc
    from concourse.masks import make_identity

    _patch_act_tables()

    fp32 = mybir.dt.float32
    bf16 = mybir.dt.bfloat16
    AF = mybir.ActivationFunctionType
    ALU = mybir.AluOpType
    AX = mybir.AxisListType

    nc = bacc.Bacc("TRN2", target_bir_lowering=False, debug=False)
    x1 = nc.declare_dram_parameter("x1", [BC, K], fp32, isOutput=False)
    x2 = nc.declare_dram_parameter("x2", [BC, K], fp32, isOutput=False)
    m1 = nc.declare_dram_parameter("m1", [BC, N], fp32, isOutput=False)
    m2 = nc.declare_dram_parameter("m2", [BC, N], fp32, isOutput=False)
    g = nc.declare_dram_parameter("g", [BC, N * S], fp32, isOutput=False)
    out = nc.declare_dram_parameter("out", [BC, S], fp32, isOutput=True)

    # [b, p, c]: p = n-octet on partitions, c = i*32 + s (1KB contiguous)
    g3 = g.rearrange("b (p c) -> b p c", p=P)

    with (
        tile.TileContext(nc) as tc,
        tc.tile_pool(name="const", bufs=1) as constp,
        tc.tile_pool(name="prep", bufs=2) as prep,
        tc.tile_pool(name="wpool", bufs=NT) as wpool,
        tc.tile_pool(name="gate", bufs=8) as gatep,
        tc.tile_pool(name="eg", bufs=2) as egp,
        tc.tile_pool(name="small", bufs=2) as small,
        tc.tile_pool(name="tsmall", bufs=2) as tsmall,
        tc.tile_pool(name="psum_t", bufs=2, space="PSUM") as psum_t,
        tc.tile_pool(name="psum_o", bufs=3, space="PSUM") as psum_o,
    ):
        ident = constp.tile([P, P], fp32)
        make_identity(nc, ident[:])
        # ones row for the C1 partition-broadcast matmul (K=1 contraction)
        ones1 = constp.tile([1, K], fp32)
        nc.vector.memset(ones1[:], 1.0)
        c1all = constp.tile([P, NT], fp32)   # C1 per batch row, col = tile
        c1S = constp.tile([K, BC], fp32)     # C1 broadcast to [32 s, 512 b]

        w_all = {}

        def emit_prep(t):
            bs = t * P
            xt1 = prep.tile([P, K], fp32, tag="x1", name=f"xt1_{t}")
            nc.sync.dma_start(out=xt1[:], in_=x1[bs : bs + P, :])
            xt2 = prep.tile([P, K], fp32, tag="x2", name=f"xt2_{t}")
            nc.sync.dma_start(out=xt2[:], in_=x2[bs : bs + P, :])
            mt1 = prep.tile([P, K, K], fp32, tag="m1", name=f"mt1_{t}")
            nc.sync.dma_start(
                out=mt1[:], in_=m1[bs : bs + P, :].rearrange("p (i j) -> p i j", j=K)
            )
            mt2 = prep.tile([P, K, K], fp32, tag="m2", name=f"mt2_{t}")
            nc.sync.dma_start(
                out=mt2[:], in_=m2[bs : bs + P, :].rearrange("p (i j) -> p i j", j=K)
            )

            raws = []
            minrs = []
            for xt, mt, tagn in ((xt1, mt1, "1"), (xt2, mt2, "2")):
                d = prep.tile([P, K, K], fp32, tag="d" + tagn, name=f"d{tagn}_{t}")
                nc.vector.tensor_sub(
                    d[:], mt[:], xt[:].unsqueeze(1).broadcast_to([P, K, K])
                )
                nc.vector.tensor_mul(d[:], d[:], d[:])
                raw = prep.tile([P, K], fp32, tag="raw" + tagn, name=f"raw{tagn}_{t}")
                nc.vector.tensor_reduce(raw[:], d[:], axis=AX.X, op=ALU.add)
                minr = prep.tile([P, 1], fp32, tag="minr" + tagn, name=f"minr{tagn}_{t}")
                nc.vector.tensor_reduce(minr[:], raw[:], axis=AX.X, op=ALU.min)
                nc.vector.tensor_sub(raw[:], raw[:], minr[:].broadcast_to([P, K]))
                raws.append(raw)
                minrs.append(minr)

            rawp = prep.tile([P, K, K], fp32, tag="rawp", name=f"rawp_{t}")
            nc.vector.tensor_add(
                rawp[:],
                raws[1][:].unsqueeze(2).broadcast_to([P, K, K]),
                raws[0][:].unsqueeze(1).broadcast_to([P, K, K]),
            )
            rawp_f = rawp[:].rearrange("p i j -> p (i j)")

            # C1[b] = -0.5*(minr1+minr2) + BCONST into column t of c1all
            c1t = prep.tile([P, 1], fp32, tag="c1", name=f"c1_{t}")
            nc.vector.tensor_add(c1t[:], minrs[0][:], minrs[1][:])
            nc.vector.tensor_scalar(
                out=c1all[:, t : t + 1], in0=c1t[:], scalar1=-0.5, scalar2=BCONST,
                op0=ALU.mult, op1=ALU.add,
            )

            # stationary weights: w_t[p, i, q, m] = exp(-0.5*raw'[b=32q+m, 8p+i])
            # for m < 32; columns 32..63 of each quarter group = 1.0 (the
            # S0 ones block, so one 64-col stationary serves S1 and S0)
            w_t = wpool.tile([P, NI, NQ, 2 * K], bf16, tag="w", bufs=NT, name=f"w_{t}")
            nc.vector.memset(w_t[:], 1.0)
            for hb in range(2):
                pt = psum_t.tile([P, 4 * P], fp32, tag="pt", name=f"pt_{t}_{hb}")
                for ii in range(4):
                    i = 4 * hb + ii
                    nc.tensor.transpose(
                        pt[:, ii * P : (ii + 1) * P],
                        _strided_cols(bass, rawp_f, i, NI, P),
                        ident[:],
                    )
                nc.scalar.activation(
                    w_t[:, 4 * hb : 4 * hb + 4, :, 0:32],
                    pt[:].rearrange("p (ii q m) -> p ii q m", ii=4, m=32),
                    AF.Exp,
                    scale=-0.5,
                )
            w_all[t] = w_t

        def emit_c1_broadcast():
            # c1all [128 b, 4 t] -> c1S [32 s, 512 b], all on-chip:
            # 4 transposes lay C1 out as one [1, 512] PSUM row; a K=1
            # matmul against the ones stationary broadcasts it to 32
            # partitions.  Reuses the prep-transpose PSUM pool (its ring
            # is idle once the preps are done) to stay within 8 banks.
            c1Pt = psum_t.tile([P, 4 * P], fp32, tag="pt", name="c1P")
            c1P = c1Pt[0:1, :]
            for t in range(NT):
                nc.tensor.transpose(
                    c1P[:, t * P : (t + 1) * P], c1all[:, t : t + 1], ident[:]
                )
            c1row = constp.tile([1, BC], fp32)
            nc.vector.tensor_copy(c1row[:], c1P[:])
            c1bPt = psum_t.tile([P, 4 * P], fp32, tag="pt", name="c1bP")
            c1bP = c1bPt[0:K, :]
            nc.tensor.matmul(c1bP[:, :], ones1[:], c1row[:], start=True, stop=True)
            nc.vector.tensor_copy(c1S[:], c1bP[:, :])

        def emit_quarter(t, q, last=False):
            # Gate streams in four 8-row sub-chunks (1MB DMA completion
            # units) so ACT never waits on a whole 2MB transfer and DMA
            # hiccups smooth out.  The two 512-col matmuls per octet share
            # one 64-col stationary (32 w cols + 32 ones cols); for the
            # final quarter the matmuls run per 8-row sub-chunk (256 cols)
            # to shorten the tail after the last gate byte.
            bq = t * P + q * 32
            w_t = w_all[t]
            gts = []
            for sub in range(4):
                bh = bq + sub * 8
                gt = gatep.tile(
                    [P, 8, NI, S], bf16, tag="gt", bufs=14,
                    name=f"gt_{t}_{q}_{sub}",
                )
                nc.gpsimd.dma_start(
                    out=gt[:], in_=g3[bh : bh + 8, :, :].transpose([1, 0, 2])
                )
                gts.append(gt)
            po = psum_o.tile([2 * K, 2, 512], fp32, tag="po", name=f"po_{t}_{q}")
            eg = egp.tile(
                [P, 32, NI, S], bf16, tag="eg", bufs=3, name=f"eg_{t}_{q}"
            )

            def emit_sub(sub):
                nc.scalar.activation(
                    eg[:, 8 * sub : 8 * sub + 8, :, :], gts[sub][:], AF.Exp
                )
                if last:
                    for i in range(NI):
                        nc.tensor.matmul(
                            po[:, sub // 2, 256 * (sub % 2) : 256 * (sub % 2) + 256],
                            w_t[:, i, q, :],
                            eg[:, 8 * sub : 8 * sub + 8, i, :],
                            start=(i == 0),
                            stop=(i == NI - 1),
                        )

            def emit_mms(h):
                if last:
                    return
                for i in range(NI):
                    nc.tensor.matmul(
                        po[:, h, :],
                        w_t[:, i, q, :],
                        eg[:, 16 * h : 16 * h + 16, i, :],
                        start=(i == 0),
                        stop=(i == NI - 1),
                    )

            return po, bq, emit_sub, emit_mms

        def emit_evac(po, bq, tag):
            # po rows 0..31 = S1 for batch row m (column block m), rows
            # 32..63 = S0 (ones block).  Ln both; the S0 block is moved to
            # partitions 0..31 with a small SBUF->SBUF DMA (engines cannot
            # cross lanes) before the 32x32 block transposes.
            lg = small.tile([2 * K, 1024], fp32, tag="lg", name=f"lg_{tag}")
            nc.scalar.activation(lg[:], po[:], AF.Ln)
            lgS0 = small.tile([K, 1024], fp32, tag="lgS0", name=f"lgS0_{tag}")
            nc.sync.dma_start(out=lgS0[:], in_=lg[K : 2 * K, :])
            T1 = tsmall.tile([K, 1024], fp32, tag="T1", name=f"T1_{tag}")
            nc.vector.transpose(T1[:], lg[0:K, :])
            T0 = tsmall.tile([K, 1024], fp32, tag="T0", name=f"T0_{tag}")
            nc.vector.transpose(T0[:], lgS0[:])
            res = small.tile([K, 32], fp32, tag="res", name=f"res_{tag}")
            nc.vector.tensor_sub(
                res[:],
                _strided_cols(bass, T1[:], 0, 33, 32),
                _strided_cols(bass, T0[:], 0, 32, 32),
            )
            nc.vector.tensor_add(res[:], res[:], c1S[:, bq : bq + 32])
            oq = small.tile([32, S], fp32, tag="oq", name=f"oq_{tag}")
            nc.vector.transpose(oq[:], res[:])
            nc.sync.dma_start(out=out[bq : bq + 32, :], in_=oq[:])

        # ---- emission ----
        for t in range(NT):
            emit_prep(t)
        emit_c1_broadcast()

        pending = None
        for step in range(NT * NQ):
            t, q = divmod(step, NQ)
            last = step == NT * NQ - 1
            po, bq, emit_sub, emit_mms = emit_quarter(t, q, last=last)
            emit_sub(0)
            emit_sub(1)
            if pending is not None:
                emit_evac(*pending)
            emit_mms(0)
            emit_sub(2)
            emit_sub(3)
            emit_mms(1)
            pending = (po, bq, f"{t}_{q}")
        emit_evac(*pending)

    nc.compile()
    return nc


def _get_nc():
    if "nc" not in _cache:
        _cache["nc"] = _build()
    return _cache["nc"]


def kernel(x, means1, means2, gate_params, scope1, scope2):
    from concourse.bass_utils import run_bass_kernel_spmd

    x = np.asarray(x, dtype=np.float32)
    means1 = np.ascontiguousarray(np.asarray(means1, dtype=np.float32))
    means2 = np.ascontiguousarray(np.asarray(means2, dtype=np.float32))
    gp = np.ascontiguousarray(
        np.asarray(gate_params, dtype=np.float32).reshape(B, N * S)
    )
    xs1 = np.ascontiguousarray(x[:, np.asarray(scope1)])
    xs2 = np.ascontiguousarray(x[:, np.asarray(scope2)])

    nc = _get_nc()
    in_maps = []
    for c in range(NCORES):
        sl = slice(c * BC, (c + 1) * BC)
        in_maps.append(
            {
                "x1": xs1[sl],
                "x2": xs2[sl],
                "m1": means1[sl],
                "m2": means2[sl],
                "g": gp[sl],
            }
        )
    res = run_bass_kernel_spmd(nc, in_maps, core_ids=list(range(NCORES)))
    return np.concatenate([res.results[c]["out"] for c in range(NCORES)], axis=0)
